# revision 26
# baseline (speedup 1.0000x reference)
"""Trainium2 Bass kernel: fused bmm+decay+reduce attention scorer.

Computes, for full inputs
    self_attn  [N=16, M=100, EMB=128] f32
    self_delta [N=16, M=100, L=10000, D=4] f32
    emb_table  [L+1=10001, EMB=128] f32
    value_w    [M=100] f32
the output
    out[n, l] = sum_m value_w[m] * (sum_d self_delta[n,m,l,d]) * (emb_table[1+l] . self_attn[n,m])
of shape [16, 10000] f32 (matches the reference jnp einsum chain).

Sharding: the candidate/location axis L is split 8 ways (1250 locations per
core); every core handles all 16 batch rows for its location range.  This
keeps the dominant stream (self_delta) un-replicated and only replicates the
small attn/value tensors; the embedding table is row-sharded.

The delta stream is staged host-side in bf16 (the 2e-2 rel-err budget dwarfs
bf16's ~5e-3; fp16 measured ~20% slower on both PE and DVE) and in
d-plane-major order raw[r, d*1250 + l], r = n*100 + m, so every DVE op runs
dense step-1 16-bit operands = 2x perf mode:
  - the D-reduction is two dense pairwise adds,
  - the decay multiply is one dense bf16 mul against S staged in SBUF.
Per 128-row tile: PE computes S[p,l] = emb[l].attn[r(p)] (bf16 matmul, EMB
on the contraction partitions) into a 3-bank PSUM tile; the otherwise-idle
ScalarE evacuates S to SBUF as bf16 (freeing the DVE from 1x-rate PSUM
reads); DVE folds D and multiplies; and a vw-scaled one-hot stationary
operand vwoh[p, j] = vw[m(r)] * (n(r) == j) routes each partition's
contribution to its own output row, accumulating all 16 rows in PSUM across
tiles.  The acc matmuls are software-pipelined one tile behind the S
matmuls so the PE never idles on the S->ScalarE->mul chain.

The kernel is HBM-bound: the raw stream (16 MB/core) runs gapless at the
~322-390 GB/s the core wins of its shared HBM stack.  Hence no row padding:
the 64-row remainder tile DMAs only its real rows (partitions 64-127 keep
the buffer slot's previous finite contents, masked by zero vwoh columns).  raw DMA buffers are capped at
6: deeper prefetch makes the DMA front-run compute and the SBUF write
pressure inflates every engine's op durations ~20% (measured).  The remainder
tile is processed last, fetched as staged half/quarter/quarter DMAs with a
restructured (d0+d1)+(d2+d3) fold per l-half, so only a quarter tile of DVE
work trails the stream's final byte; the first tile is likewise fetched in
two halves so the DVE pipeline starts ~2us earlier, and the final PSUM
evacuation runs as three back-to-back VectorE copies (a DVE/ScalarE mix
was tried: Tile serializes the engines with sem handoffs anyway, so the
mix only added cross-engine latency; ~0.8us recoverable if a future rework
truly parallelizes the copies without a scheduler-wide reshuffle).  The raw stream and
output go through SWDGE; constants use the scalar HWDGE ring.
"""

import ml_dtypes
import numpy as np

import concourse.mybir as mybir
import concourse.tile as tile
from concourse import bacc
from concourse.bass_utils import run_bass_kernel_spmd

BF16 = ml_dtypes.bfloat16

N, M, L, EMB, D = 16, 100, 10000, 128, 4
NCORES = 8
LSH = L // NCORES  # 1250 locations per core
R = N * M  # 1600 flattened (n, m) rows
P = 128
NTILE = (R + P - 1) // P  # 13 tiles; the last holds 64 real rows
ROW0 = [t * P for t in range(NTILE - 1)] + [R - 64]  # tile row starts; the last tile holds only the 64 remainder rows
TILE_ORDER = list(range(NTILE))  # remainder tile last: half-size DMA shortens the stream tail
# matmul moving-operand chunks: <=512 (PSUM bank), bank-aligned offsets
CHUNKS = [(0, 512), (512, 512), (1024, 226)]
HALF = LSH // 2  # 625
DT16 = mybir.dt.bfloat16
FP32 = mybir.dt.float32

_NC_CACHE = {}


def _build_nc():
    nc = bacc.Bacc(
        "TRN2", target_bir_lowering=False, debug=False, num_devices=NCORES
    )
    # raw[r, d*LSH + l] = self_delta[n, m, lo+l, d]  (bf16, d-plane major)
    raw_d = nc.dram_tensor("raw", [R, D * LSH], DT16, kind="ExternalInput").ap()
    embT_d = nc.dram_tensor("embT", [EMB, LSH], DT16, kind="ExternalInput").ap()
    attnT_d = nc.dram_tensor("attnT", [EMB, R], DT16, kind="ExternalInput").ap()
    vwoh_d = nc.dram_tensor("vwoh", [P, NTILE * N], DT16, kind="ExternalInput").ap()
    out_d = nc.dram_tensor("out", [N, LSH], FP32, kind="ExternalOutput").ap()

    with tile.TileContext(nc) as tc:
        with (
            tc.tile_pool(name="const", bufs=1) as cpool,
            tc.tile_pool(name="raws", bufs=6) as rpool,
            tc.tile_pool(name="a1p", bufs=2) as a1pool,
            tc.tile_pool(name="work", bufs=2) as wpool,
            tc.tile_pool(name="spsum", bufs=1, space="PSUM") as spool,
            tc.tile_pool(name="apsum", bufs=1, space="PSUM") as apool,
        ):
            embT = cpool.tile([EMB, LSH], DT16, tag="embT")
            nc.scalar.dma_start(out=embT, in_=embT_d)
            attnT = cpool.tile([EMB, R], DT16, tag="attnT")
            nc.scalar.dma_start(out=attnT, in_=attnT_d)
            vwoh = cpool.tile([P, NTILE * N], DT16, tag="vwoh")
            nc.scalar.dma_start(out=vwoh, in_=vwoh_d)

            # out accumulator rows n=0..15, 3 PSUM banks, lives whole kernel
            acc = apool.tile([N, LSH], FP32, tag="acc")

            pending = None  # (pt, t) of the previous tile, acc-mm'd next iter

            def emit_acc(pt, t, *, first, last):
                for c0, w in CHUNKS:
                    nc.tensor.matmul(
                        acc[:, c0 : c0 + w],
                        vwoh[:, t * N : (t + 1) * N],
                        pt[:, c0 : c0 + w],
                        start=first,
                        stop=last,
                    )

            for ti, t in enumerate(TILE_ORDER):
                last = ti == NTILE - 1
                raw = rpool.tile([P, D * LSH], DT16, tag="raw")
                if last:
                    # only the 64 remainder rows, as staged half/quarter/
                    # quarter DMAs so just a quarter of the tile's DVE work
                    # trails the final byte of the stream.  Partitions 64-127
                    # keep the slot's previous tile: finite values, masked by
                    # zero vwoh columns.
                    rv = raw.rearrange("p (d l) -> p d l", d=D)[0:64]
                    rd3 = raw_d.rearrange("r (d l) -> r d l", d=D)
                    rr = slice(ROW0[t], ROW0[t] + 64)
                    nc.gpsimd.dma_start(out=rv[:, 0:2], in_=rd3[rr, 0:2])
                    nc.gpsimd.dma_start(
                        out=rv[:, 2:4, 0:HALF], in_=rd3[rr, 2:4, 0:HALF]
                    )
                    nc.gpsimd.dma_start(
                        out=rv[:, 2:4, HALF:LSH], in_=rd3[rr, 2:4, HALF:LSH]
                    )
                elif ti == 0:
                    # first tile as two half DMAs: the d-fold starts ~2us
                    # earlier, shortening the pipeline ramp
                    nc.gpsimd.dma_start(
                        out=raw[:, 0 : 2 * LSH],
                        in_=raw_d[ROW0[t] : ROW0[t] + P, 0 : 2 * LSH],
                    )
                    nc.gpsimd.dma_start(
                        out=raw[:, 2 * LSH : 4 * LSH],
                        in_=raw_d[ROW0[t] : ROW0[t] + P, 2 * LSH : 4 * LSH],
                    )
                else:
                    nc.gpsimd.dma_start(out=raw, in_=raw_d[ROW0[t] : ROW0[t] + P])

                # S[p, l] = attn[r(p)] . emb_cand[lo+l], bf16 in, fp32 PSUM out
                rows = 64 if last else P
                s_ps = spool.tile([P, LSH], FP32, tag="s")
                for c0, w in CHUNKS:
                    nc.tensor.matmul(
                        s_ps[:rows, c0 : c0 + w],
                        attnT[:, ROW0[t] : ROW0[t] + rows],
                        embT[:, c0 : c0 + w],
                        start=True,
                        stop=True,
                    )
                # previous tile's output accumulation rides behind this
                # tile's S matmuls in the PE stream
                if pending is not None:
                    emit_acc(*pending, first=(ti == 1), last=False)

                # ScalarE evacuates S to SBUF as bf16 (2x-mode DVE operand)
                s_sb = wpool.tile([P, LSH], DT16, tag="ssb")
                nc.scalar.copy(out=s_sb, in_=s_ps)

                # delta[p,l] = sum_d raw[p,l,d]: dense 2x-mode adds
                a1 = a1pool.tile([P, 2 * LSH], DT16, tag="a1")
                a2 = wpool.tile([P, LSH], DT16, tag="a2")
                pt = wpool.tile([P, LSH], DT16, tag="pt")
                if last:
                    # (d0+d1) after the half DMA, then +(d2+d3) per l-half
                    # as each quarter DMA lands
                    nc.vector.tensor_add(
                        out=a1[:, 0:LSH], in0=raw[:, 0:LSH], in1=raw[:, LSH : 2 * LSH]
                    )
                    for h0, h1 in ((0, HALF), (HALF, LSH)):
                        nc.vector.tensor_add(
                            out=a1[:, LSH + h0 : LSH + h1],
                            in0=raw[:, 2 * LSH + h0 : 2 * LSH + h1],
                            in1=raw[:, 3 * LSH + h0 : 3 * LSH + h1],
                        )
                        nc.vector.tensor_add(
                            out=a2[:, h0:h1],
                            in0=a1[:, h0:h1],
                            in1=a1[:, LSH + h0 : LSH + h1],
                        )
                        nc.vector.tensor_mul(
                            out=pt[:, h0:h1], in0=a2[:, h0:h1], in1=s_sb[:, h0:h1]
                        )
                elif ti == 0:
                    # (d0+d1) after the first half lands, then +(d2+d3)
                    nc.vector.tensor_add(
                        out=a1[:, 0:LSH], in0=raw[:, 0:LSH], in1=raw[:, LSH : 2 * LSH]
                    )
                    nc.vector.tensor_add(
                        out=a1[:, LSH : 2 * LSH],
                        in0=raw[:, 2 * LSH : 3 * LSH],
                        in1=raw[:, 3 * LSH : 4 * LSH],
                    )
                    nc.vector.tensor_add(
                        out=a2, in0=a1[:, 0:LSH], in1=a1[:, LSH : 2 * LSH]
                    )
                else:
                    nc.vector.tensor_add(
                        out=a1, in0=raw[:, 0 : 2 * LSH], in1=raw[:, 2 * LSH : 4 * LSH]
                    )
                    nc.vector.tensor_add(
                        out=a2, in0=a1[:, 0:LSH], in1=a1[:, LSH : 2 * LSH]
                    )
                if not last:
                    # Pt[p, l] = delta[p, l] * S[p, l]
                    nc.vector.tensor_mul(out=pt, in0=a2, in1=s_sb)
                pending = (pt, t)

            emit_acc(*pending, first=False, last=True)

            out_sb = cpool.tile([N, LSH], FP32, tag="out_sb")
            for c0, w in CHUNKS:
                nc.vector.tensor_copy(out=out_sb[:, c0 : c0 + w], in_=acc[:, c0 : c0 + w])
            nc.gpsimd.dma_start(out=out_d, in_=out_sb)

    nc.compile()
    return nc


def _get_nc():
    if "nc" not in _NC_CACHE:
        _NC_CACHE["nc"] = _build_nc()
    return _NC_CACHE["nc"]


def _prep_in_maps(self_attn, self_delta, emb_table, value_w):
    self_attn = np.asarray(self_attn, dtype=np.float32)
    self_delta = np.asarray(self_delta, dtype=np.float32)
    emb_table = np.asarray(emb_table, dtype=np.float32)
    value_w = np.asarray(value_w, dtype=np.float32)

    embT_full = emb_table[1 : L + 1].T.astype(BF16)  # [EMB, L]

    # column r = n*M + m of attnT holds attn[n, m, :]
    attnT = self_attn.transpose(2, 0, 1).reshape(EMB, R).astype(BF16)

    # vwoh[p, t*N + j] = vw[m(r)] * (n(r) == j),  r = ROW0[t] + p; the
    # overlap tile's first 64 rows are already counted by tile 11 -> zero
    vwoh = np.zeros((P, NTILE * N), dtype=BF16)
    for t in range(NTILE):
        pmax = 64 if t == NTILE - 1 else P
        for p in range(pmax):
            r = ROW0[t] + p
            vwoh[p, t * N + (r // M)] = value_w[r % M]

    in_maps = []
    for c in range(NCORES):
        lo = c * LSH
        # raw[r, d*LSH + l] = self_delta[n, m, lo+l, d]
        raw_c = np.empty((R, D * LSH), dtype=BF16)
        raw_c.reshape(N, M, D, LSH)[...] = self_delta[
            :, :, lo : lo + LSH, :
        ].transpose(0, 1, 3, 2)
        in_maps.append(
            {
                "raw": raw_c,
                "embT": np.ascontiguousarray(embT_full[:, lo : lo + LSH]),
                "attnT": attnT,
                "vwoh": vwoh,
            }
        )
    return in_maps


def _run(inputs, **spmd_kwargs):
    in_maps = _prep_in_maps(
        inputs["self_attn"], inputs["self_delta"], inputs["emb_table"], inputs["value_w"]
    )
    res = run_bass_kernel_spmd(
        _get_nc(), in_maps, core_ids=list(range(NCORES)), **spmd_kwargs
    )
    out = np.concatenate([r["out"] for r in res.results], axis=1)  # [N, L]
    return out, res


def kernel(**inputs) -> np.ndarray:
    out, _ = _run(inputs)
    return out
